# revision 1
# baseline (speedup 1.0000x reference)
"""MoE feed-forward (shared + top-2 of 8 routed experts), expert-parallel
across 8 trn2 cores.

Sharding strategy (per the spec's expert-parallel hint): the stacked expert
weights [E,d,f] are sharded along E — core c owns expert c. Token dispatch/
combine happens at the shard/unshard boundary on the host: while building
per-core inputs, the host runs the (tiny, 0.1% of FLOPs) router
(softmax -> top-2 -> renormalize) and gathers each expert's tokens into that
core's input shard, padded to a uniform capacity CE (SPMD requires one
program). The shared expert is token-parallel: core c also runs the shared
FFN for tokens [512c, 512(c+1)). Unshard scatters the (device-scaled) expert
rows back and sums with the shared rows; each expert's token list has unique
token ids, so fancy-index += is exact.

Device program per core (identical on all 8):
  - routed:  h^T = Wg^T xg^T, u^T = Wu^T xg^T (bf16, fp32 PSUM), g = silu(h)*u
             y^T = Wd^T g^T   [CE tokens]
  - shared:  same, 512 tokens
Activations keep tokens in the free dim throughout (transposed layout): every
matmul's stationary operand is a natural [128,128] weight tile that streams
all tokens back-to-back (weight loads hide under the previous stream), and
PSUM groups are 3x384 tokens. The host applies the per-token gate weight and
un-transposes during unshard. All DMA is split into <=256KB per-chunk
transfers on the SP HWDGE ring (large single transfers serialize; the ACT
ring proved both slower and rece-prone here).
"""

import numpy as np

E = 8          # routed experts
D = 1024       # hidden
F = 1024       # intermediate
B, S = 2, 2048
T = B * S      # 4096 tokens
NCORES = 8
TCS = T // NCORES  # 512 shared-expert tokens per core
P = 128
DK = D // P    # 8 contraction chunks over D
FT = F // P    # 8 f tiles

_CACHE: dict = {}
_ROUTING: dict = {}


def _psum_groups(n):
    # near-uniform groups of <=512 tokens (uniform sizes keep the weight-load
    # fully hidden under each stream)
    k = -(-n // 512)
    q = -(-n // k)
    out = []
    o = 0
    while o < n:
        g = min(q, n - o)
        out.append((o, g))
        o += g
    return out


def _build_nc(ce, reps=1, loop_reps=0):
    import concourse.mybir as mybir
    import concourse.tile as tile
    from concourse import bacc
    from concourse.bass import ts, ds

    dt = mybir.dt
    f32 = dt.float32
    bf16 = dt.bfloat16
    Alu = mybir.AluOpType
    Act = mybir.ActivationFunctionType


    nc = bacc.Bacc("TRN2", target_bir_lowering=False, debug=False,
                   num_devices=NCORES)

    xr_d = nc.dram_tensor("xrT", [P, DK, ce], bf16, kind="ExternalInput").ap()
    xs_d = nc.dram_tensor("xsT", [P, DK, TCS], bf16, kind="ExternalInput").ap()
    wg_d = nc.dram_tensor("wg", [2, P, DK, F], bf16, kind="ExternalInput").ap()
    wu_d = nc.dram_tensor("wu", [2, P, DK, F], bf16, kind="ExternalInput").ap()
    wd_d = nc.dram_tensor("wd", [2, P, FT, D], bf16, kind="ExternalInput").ap()
    y_d = nc.dram_tensor("y", [DK, P, ce + TCS], bf16,
                         kind="ExternalOutput").ap()

    with tile.TileContext(nc) as tc:
        with (
            tc.tile_pool(name="const", bufs=1) as constp,
            tc.tile_pool(name="wgp", bufs=2) as wgp,
            tc.tile_pool(name="wup", bufs=2) as wup,
            tc.tile_pool(name="wdp", bufs=2) as wdp,
            tc.tile_pool(name="gp", bufs=2) as gp,
            tc.tile_pool(name="yp", bufs=4) as yp,
            tc.tile_pool(name="php", bufs=8, space="PSUM") as php,
        ):
          import contextlib
          loop_cm = (tc.For_i(0, loop_reps, 1) if loop_reps
                     else contextlib.nullcontext())
          with loop_cm:
           for _rep in range(reps):
              xr = constp.tile([P, DK, ce], bf16)
              for dk in range(DK):
                  if dk == 0:
                      # split dk0 by psum group: the very first matmul only
                      # needs tokens [0, g0) — don't stall PE on the full chunk
                      for (go, gl) in _psum_groups(ce):
                          nc.sync.dma_start(xr[:, 0, ds(go, gl)],
                                            xr_d[:, 0, ds(go, gl)])
                  else:
                      nc.sync.dma_start(xr[:, dk], xr_d[:, dk])
              xs = constp.tile([P, DK, TCS], bf16)
              for dk in range(DK):
                  nc.sync.dma_start(xs[:, dk], xs_d[:, dk])

              def ffn_gu(e, xa, ntok):
                  # weights for this part, split into per-chunk DMAs
                  wg_sb = wgp.tile([P, DK, F], bf16, tag="wg")
                  for dk in range(DK):
                      if dk == 0:
                          for ft in range(FT):
                              nc.sync.dma_start(wg_sb[:, 0, ts(ft, P)],
                                                wg_d[e, :, 0, ts(ft, P)])
                      else:
                          nc.sync.dma_start(wg_sb[:, dk], wg_d[e, :, dk])
                  wu_sb = wup.tile([P, DK, F], bf16, tag="wu")
                  for dk in range(DK):
                      if dk == 0:
                          for ft in range(FT):
                              nc.sync.dma_start(wu_sb[:, 0, ts(ft, P)],
                                                wu_d[e, :, 0, ts(ft, P)])
                      else:
                          nc.sync.dma_start(wu_sb[:, dk], wu_d[e, :, dk])
                  wd_sb = wdp.tile([P, FT, D], bf16, tag="wd")
                  for fk in range(FT):
                      nc.sync.dma_start(wd_sb[:, fk], wd_d[e, :, fk])

                  g_sb = gp.tile([P, FT, ntok], bf16, tag="g")
                  groups = _psum_groups(ntok)
                  for ft in range(FT):
                      # dk outer: each [128,128] weight tile streams all ntok
                      # rows (across psum groups) back-to-back
                      phs = []
                      for gi in range(len(groups)):
                          ph = php.tile([P, groups[gi][1]], f32, tag="ph")
                          phs.append(ph)
                      for dk in range(DK):
                          for gi, (go, gl) in enumerate(groups):
                              nc.tensor.matmul(
                                  phs[gi][:], wg_sb[:, dk, ts(ft, P)],
                                  xa[:, dk, ds(go, gl)],
                                  start=(dk == 0), stop=(dk == DK - 1),
                              )
                      pus = []
                      for gi in range(len(groups)):
                          pu = php.tile([P, groups[gi][1]], f32, tag="ph")
                          pus.append(pu)
                      for dk in range(DK):
                          for gi, (go, gl) in enumerate(groups):
                              nc.tensor.matmul(
                                  pus[gi][:], wu_sb[:, dk, ts(ft, P)],
                                  xa[:, dk, ds(go, gl)],
                                  start=(dk == 0), stop=(dk == DK - 1),
                              )
                      for gi, (go, gl) in enumerate(groups):
                          nc.scalar.activation(g_sb[:, ft, ds(go, gl)],
                                               phs[gi][:], Act.Silu)
                          nc.vector.tensor_tensor(g_sb[:, ft, ds(go, gl)],
                                                  g_sb[:, ft, ds(go, gl)],
                                                  pus[gi][:], Alu.mult)

                  return g_sb, wd_sb, groups

              def ffn_down(state, ntok, col0):
                  g_sb, wd_sb, groups = state
                  # transposed down: y^T[dch, :, tok] — each Wd lhsT tile
                  # streams all ntok rows (across groups) so its weight load
                  # hides fully under the previous stream
                  for dch in range(DK):
                      pys = []
                      for gi in range(len(groups)):
                          py = php.tile([P, groups[gi][1]], f32, tag="ph")
                          pys.append(py)
                      for fk in range(FT):
                          for gi, (go, gl) in enumerate(groups):
                              nc.tensor.matmul(
                                  pys[gi][:], wd_sb[:, fk, ds(dch * P, P)],
                                  g_sb[:, fk, ds(go, gl)],
                                  start=(fk == 0), stop=(fk == FT - 1),
                              )
                      for gi, (go, gl) in enumerate(groups):
                          ysb = yp.tile([P, groups[gi][1]], bf16, tag="y")
                          nc.vector.tensor_copy(ysb[:], pys[gi][:])
                          nc.sync.dma_start(
                              y_d[dch, :, ds(col0 + go, gl)], ysb[:])

              st0 = ffn_gu(0, xr, ce)
              st1 = ffn_gu(1, xs, TCS)
              ffn_down(st0, ce, 0)
              ffn_down(st1, TCS, ce)

    nc.compile()
    return nc


def _get_nc(reps=1, loop_reps=0):
    ce = _ROUTING["ce"]
    key = (ce, reps, loop_reps)
    if key not in _CACHE:
        _CACHE[key] = _build_nc(ce, reps, loop_reps)
    return _CACHE[key]


def _route(x, gate_w):
    """Host router: softmax -> top-2 (jax top_k tie order) -> renormalize."""
    xf = np.asarray(x, np.float32).reshape(T, D)
    logits = xf @ np.asarray(gate_w, np.float32)
    m = logits.max(-1, keepdims=True)
    q = np.exp(logits - m)
    gate = q / q.sum(-1, keepdims=True)
    order = np.argsort(-gate, axis=-1, kind="stable")
    topi = order[:, :2]
    topw = np.take_along_axis(gate, topi, axis=-1)
    topw = topw / (topw.sum(-1, keepdims=True) + 1e-20)
    return xf, topi, topw


def make_in_maps(x, gate_w, sw_gate, sw_up, sw_down, ew_gate, ew_up, ew_down):
    import ml_dtypes
    bf16 = ml_dtypes.bfloat16

    xf, topi, topw = _route(x, gate_w)

    idxs, ws = [], []
    for e in range(E):
        sel = np.nonzero(topi == e)
        idxs.append(sel[0].astype(np.int64))          # token ids, sorted
        ws.append(topw[sel].astype(np.float32))
    counts = [len(i) for i in idxs]
    ce = max(max(counts), 1)
    _ROUTING.update(ce=ce, idxs=idxs, counts=counts, ws=ws)

    sw = [np.asarray(a, np.float32) for a in (sw_gate, sw_up, sw_down)]
    ew = [np.asarray(a, np.float32) for a in (ew_gate, ew_up, ew_down)]

    def prep_gu(w):   # [D, F] -> [128, DK, F] bf16
        return np.ascontiguousarray(
            w.reshape(DK, P, F).transpose(1, 0, 2).astype(bf16))

    def prep_d(w):    # [F, D] -> [128, FT, D] bf16
        return np.ascontiguousarray(
            w.reshape(FT, P, D).transpose(1, 0, 2).astype(bf16))

    def prep_x(rows, n):  # [n?, D] pad to n -> [128, DK, n] bf16
        xp = np.zeros((n, D), np.float32)
        xp[:len(rows)] = rows
        return np.ascontiguousarray(
            xp.T.reshape(DK, P, n).transpose(1, 0, 2).astype(bf16))

    sg, su, sd = (prep_gu(sw[0]), prep_gu(sw[1]), prep_d(sw[2]))

    in_maps = []
    for c in range(NCORES):
        xg = prep_x(xf[idxs[c]], ce)
        xsl = prep_x(xf[c * TCS:(c + 1) * TCS], TCS)
        in_maps.append({
            "xrT": xg, "xsT": xsl,
            "wg": np.stack([prep_gu(ew[0][c]), sg]),
            "wu": np.stack([prep_gu(ew[1][c]), su]),
            "wd": np.stack([prep_d(ew[2][c]), sd]),
        })
    return in_maps


def assemble_out(results):
    ce = _ROUTING["ce"]
    idxs, counts, ws = _ROUTING["idxs"], _ROUTING["counts"], _ROUTING["ws"]
    y = np.zeros((T, D), np.float32)
    for c in range(NCORES):
        yt = np.asarray(results[c]["y"], np.float32)   # [DK, P, ce+TCS]
        ys = yt[:, :, ce:].transpose(2, 0, 1).reshape(TCS, D)
        y[c * TCS:(c + 1) * TCS] = ys
    for c in range(NCORES):
        yt = np.asarray(results[c]["y"], np.float32)
        n = counts[c]
        yr = yt[:, :, :n].transpose(2, 0, 1).reshape(n, D)
        y[idxs[c]] += yr * ws[c][:, None]
    return y.reshape(B, S, D)


def kernel(x, gate_w, sw_gate, sw_up, sw_down, ew_gate, ew_up, ew_down):
    from concourse.bass_utils import run_bass_kernel_spmd

    in_maps = make_in_maps(x, gate_w, sw_gate, sw_up, sw_down,
                           ew_gate, ew_up, ew_down)
    nc = _get_nc()
    res = run_bass_kernel_spmd(nc, in_maps, list(range(NCORES)))
    return assemble_out(res.results)



# revision 2
# speedup vs baseline: 1.0373x; 1.0373x over previous
"""MoE feed-forward (shared + top-2 of 8 routed experts), expert-parallel
across 8 trn2 cores.

Sharding strategy (per the spec's expert-parallel hint): the stacked expert
weights [E,d,f] are sharded along E — core c owns expert c. Token dispatch/
combine happens at the shard/unshard boundary on the host: while building
per-core inputs, the host runs the (tiny, 0.1% of FLOPs) router
(softmax -> top-2 -> renormalize) and gathers each expert's tokens into that
core's input shard, padded to a uniform capacity CE (SPMD requires one
program). The shared expert is token-parallel: core c also runs the shared
FFN for tokens [512c, 512(c+1)). Unshard scatters the (device-scaled) expert
rows back and sums with the shared rows; each expert's token list has unique
token ids, so fancy-index += is exact.

Device program per core (identical on all 8):
  - routed:  h^T = Wg^T xg^T, u^T = Wu^T xg^T (bf16, fp32 PSUM), g = silu(h)*u
             y^T = Wd^T g^T   [CE tokens]
  - shared:  same, 512 tokens
Activations keep tokens in the free dim throughout (transposed layout): every
matmul's stationary operand is a natural [128,128] weight tile that streams
all tokens back-to-back (weight loads hide under the previous stream), and
PSUM groups are 3x384 tokens. The host applies the per-token gate weight and
un-transposes during unshard. All DMA is split into <=256KB per-chunk
transfers on the SP HWDGE ring (large single transfers serialize; the ACT
ring proved both slower and rece-prone here).
"""

import numpy as np

E = 8          # routed experts
D = 1024       # hidden
F = 1024       # intermediate
B, S = 2, 2048
T = B * S      # 4096 tokens
NCORES = 8
TCS = T // NCORES  # 512 shared-expert tokens per core
P = 128
DK = D // P    # 8 contraction chunks over D
FT = F // P    # 8 f tiles

_CACHE: dict = {}
_ROUTING: dict = {}


def _psum_groups(n):
    # near-uniform groups of <=512 tokens (uniform sizes keep the weight-load
    # fully hidden under each stream)
    k = -(-n // 512)
    q = -(-n // k)
    out = []
    o = 0
    while o < n:
        g = min(q, n - o)
        out.append((o, g))
        o += g
    return out


def _build_nc(ce, reps=1, loop_reps=0):
    import concourse.mybir as mybir
    import concourse.tile as tile
    from concourse import bacc
    from concourse.bass import ts, ds

    dt = mybir.dt
    f32 = dt.float32
    bf16 = dt.bfloat16
    Alu = mybir.AluOpType
    Act = mybir.ActivationFunctionType


    nc = bacc.Bacc("TRN2", target_bir_lowering=False, debug=False,
                   num_devices=NCORES)

    xr_d = nc.dram_tensor("xrT", [P, DK, ce], bf16, kind="ExternalInput").ap()
    xs_d = nc.dram_tensor("xsT", [P, DK, TCS], bf16, kind="ExternalInput").ap()
    wg_d = nc.dram_tensor("wg", [2, P, DK, F], bf16, kind="ExternalInput").ap()
    wu_d = nc.dram_tensor("wu", [2, P, DK, F], bf16, kind="ExternalInput").ap()
    wd_d = nc.dram_tensor("wd", [2, P, FT, D], bf16, kind="ExternalInput").ap()
    y_d = nc.dram_tensor("y", [DK, P, ce + TCS], bf16,
                         kind="ExternalOutput").ap()

    with tile.TileContext(nc) as tc:
        with (
            tc.tile_pool(name="const", bufs=1) as constp,
            tc.tile_pool(name="wgp", bufs=2) as wgp,
            tc.tile_pool(name="wup", bufs=2) as wup,
            tc.tile_pool(name="wdp", bufs=2) as wdp,
            tc.tile_pool(name="gp", bufs=2) as gp,
            tc.tile_pool(name="yp", bufs=4) as yp,
            tc.tile_pool(name="ydp", bufs=1) as ydp,
            tc.tile_pool(name="php", bufs=8, space="PSUM") as php,
        ):
          import contextlib
          ytiles = {}
          for dch in range(DK):
              ytiles[dch] = ydp.tile([P, TCS], bf16, tag=f"yd{dch}",
                                     name=f"ydt{dch}")

          def flush_y():
              for dch in range(DK):
                  nc.sync.dma_start(y_d[dch, :, ds(ce, TCS)],
                                    ytiles[dch][:])

          loop_cm = (tc.For_i(0, loop_reps, 1) if loop_reps
                     else contextlib.nullcontext())
          with loop_cm:
           for _rep in range(reps):
              xr = constp.tile([P, DK, ce], bf16)
              nc.sync.dma_start(xr[:], xr_d[:])
              xs = constp.tile([P, DK, TCS], bf16)
              nc.sync.dma_start(xs[:], xs_d[:])

              def ffn_gu(e, xa, ntok):
                  # weights for this part, split into per-chunk DMAs
                  wg_sb = wgp.tile([P, DK, F], bf16, tag="wg")
                  for dk in range(DK):
                      if dk == 0:
                          for ft in range(FT):
                              nc.sync.dma_start(wg_sb[:, 0, ts(ft, P)],
                                                wg_d[e, :, 0, ts(ft, P)])
                      else:
                          nc.sync.dma_start(wg_sb[:, dk], wg_d[e, :, dk])
                  wu_sb = wup.tile([P, DK, F], bf16, tag="wu")
                  for dk in range(DK):
                      if dk == 0:
                          for ft in range(FT):
                              nc.sync.dma_start(wu_sb[:, 0, ts(ft, P)],
                                                wu_d[e, :, 0, ts(ft, P)])
                      else:
                          nc.sync.dma_start(wu_sb[:, dk], wu_d[e, :, dk])
                  wd_sb = wdp.tile([P, FT, D], bf16, tag="wd")
                  for fk in range(FT):
                      nc.sync.dma_start(wd_sb[:, fk], wd_d[e, :, fk])

                  g_sb = gp.tile([P, FT, ntok], bf16, tag="g")
                  groups = _psum_groups(ntok)
                  for ft in range(FT):
                      # dk outer: each [128,128] weight tile streams all ntok
                      # rows (across psum groups) back-to-back
                      phs = []
                      for gi in range(len(groups)):
                          ph = php.tile([P, groups[gi][1]], f32, tag="ph")
                          phs.append(ph)
                      for dk in range(DK):
                          for gi, (go, gl) in enumerate(groups):
                              nc.tensor.matmul(
                                  phs[gi][:], wg_sb[:, dk, ts(ft, P)],
                                  xa[:, dk, ds(go, gl)],
                                  start=(dk == 0), stop=(dk == DK - 1),
                              )
                      pus = []
                      for gi in range(len(groups)):
                          pu = php.tile([P, groups[gi][1]], f32, tag="ph")
                          pus.append(pu)
                      for dk in range(DK):
                          for gi, (go, gl) in enumerate(groups):
                              nc.tensor.matmul(
                                  pus[gi][:], wu_sb[:, dk, ts(ft, P)],
                                  xa[:, dk, ds(go, gl)],
                                  start=(dk == 0), stop=(dk == DK - 1),
                              )
                      for gi, (go, gl) in enumerate(groups):
                          nc.scalar.activation(g_sb[:, ft, ds(go, gl)],
                                               phs[gi][:], Act.Silu)
                          nc.vector.tensor_tensor(g_sb[:, ft, ds(go, gl)],
                                                  g_sb[:, ft, ds(go, gl)],
                                                  pus[gi][:], Alu.mult)

                  return g_sb, wd_sb, groups

              def ffn_down(state, ntok, col0, delay):
                  g_sb, wd_sb, groups = state
                  # transposed down: y^T[dch, :, tok] — each Wd lhsT tile
                  # streams all ntok rows (across groups) so its weight load
                  # hides fully under the previous stream
                  for dch in range(DK):
                      pys = []
                      for gi in range(len(groups)):
                          py = php.tile([P, groups[gi][1]], f32, tag="ph")
                          pys.append(py)
                      for fk in range(FT):
                          for gi, (go, gl) in enumerate(groups):
                              nc.tensor.matmul(
                                  pys[gi][:], wd_sb[:, fk, ds(dch * P, P)],
                                  g_sb[:, fk, ds(go, gl)],
                                  start=(fk == 0), stop=(fk == FT - 1),
                              )
                      for gi, (go, gl) in enumerate(groups):
                          if delay:
                              nc.vector.tensor_copy(ytiles[dch][:], pys[gi][:])
                          else:
                              ysb = yp.tile([P, groups[gi][1]], bf16, tag="y")
                              nc.vector.tensor_copy(ysb[:], pys[gi][:])
                              nc.sync.dma_start(
                                  y_d[dch, :, ds(col0 + go, gl)], ysb[:])

              st0 = ffn_gu(0, xr, ce)
              flush_y()
              st1 = ffn_gu(1, xs, TCS)
              ffn_down(st0, ce, 0, False)
              ffn_down(st1, TCS, ce, True)

          flush_y()

    nc.compile()
    return nc


def _get_nc(reps=1, loop_reps=0):
    ce = _ROUTING["ce"]
    key = (ce, reps, loop_reps)
    if key not in _CACHE:
        _CACHE[key] = _build_nc(ce, reps, loop_reps)
    return _CACHE[key]


def _route(x, gate_w):
    """Host router: softmax -> top-2 (jax top_k tie order) -> renormalize."""
    xf = np.asarray(x, np.float32).reshape(T, D)
    logits = xf @ np.asarray(gate_w, np.float32)
    m = logits.max(-1, keepdims=True)
    q = np.exp(logits - m)
    gate = q / q.sum(-1, keepdims=True)
    order = np.argsort(-gate, axis=-1, kind="stable")
    topi = order[:, :2]
    topw = np.take_along_axis(gate, topi, axis=-1)
    topw = topw / (topw.sum(-1, keepdims=True) + 1e-20)
    return xf, topi, topw


def make_in_maps(x, gate_w, sw_gate, sw_up, sw_down, ew_gate, ew_up, ew_down):
    import ml_dtypes
    bf16 = ml_dtypes.bfloat16

    xf, topi, topw = _route(x, gate_w)

    idxs, ws = [], []
    for e in range(E):
        sel = np.nonzero(topi == e)
        idxs.append(sel[0].astype(np.int64))          # token ids, sorted
        ws.append(topw[sel].astype(np.float32))
    counts = [len(i) for i in idxs]
    ce = max(max(counts), 1)
    _ROUTING.update(ce=ce, idxs=idxs, counts=counts, ws=ws)

    sw = [np.asarray(a, np.float32) for a in (sw_gate, sw_up, sw_down)]
    ew = [np.asarray(a, np.float32) for a in (ew_gate, ew_up, ew_down)]

    def prep_gu(w):   # [D, F] -> [128, DK, F] bf16
        return np.ascontiguousarray(
            w.reshape(DK, P, F).transpose(1, 0, 2).astype(bf16))

    def prep_d(w):    # [F, D] -> [128, FT, D] bf16
        return np.ascontiguousarray(
            w.reshape(FT, P, D).transpose(1, 0, 2).astype(bf16))

    def prep_x(rows, n):  # [n?, D] pad to n -> [128, DK, n] bf16
        xp = np.zeros((n, D), np.float32)
        xp[:len(rows)] = rows
        return np.ascontiguousarray(
            xp.T.reshape(DK, P, n).transpose(1, 0, 2).astype(bf16))

    sg, su, sd = (prep_gu(sw[0]), prep_gu(sw[1]), prep_d(sw[2]))

    in_maps = []
    for c in range(NCORES):
        xg = prep_x(xf[idxs[c]], ce)
        xsl = prep_x(xf[c * TCS:(c + 1) * TCS], TCS)
        in_maps.append({
            "xrT": xg, "xsT": xsl,
            "wg": np.stack([prep_gu(ew[0][c]), sg]),
            "wu": np.stack([prep_gu(ew[1][c]), su]),
            "wd": np.stack([prep_d(ew[2][c]), sd]),
        })
    return in_maps


def assemble_out(results):
    ce = _ROUTING["ce"]
    idxs, counts, ws = _ROUTING["idxs"], _ROUTING["counts"], _ROUTING["ws"]
    y = np.zeros((T, D), np.float32)
    for c in range(NCORES):
        yt = np.asarray(results[c]["y"], np.float32)   # [DK, P, ce+TCS]
        ys = yt[:, :, ce:].transpose(2, 0, 1).reshape(TCS, D)
        y[c * TCS:(c + 1) * TCS] = ys
    for c in range(NCORES):
        yt = np.asarray(results[c]["y"], np.float32)
        n = counts[c]
        yr = yt[:, :, :n].transpose(2, 0, 1).reshape(n, D)
        y[idxs[c]] += yr * ws[c][:, None]
    return y.reshape(B, S, D)


def kernel(x, gate_w, sw_gate, sw_up, sw_down, ew_gate, ew_up, ew_down):
    from concourse.bass_utils import run_bass_kernel_spmd

    in_maps = make_in_maps(x, gate_w, sw_gate, sw_up, sw_down,
                           ew_gate, ew_up, ew_down)
    nc = _get_nc()
    res = run_bass_kernel_spmd(nc, in_maps, list(range(NCORES)))
    return assemble_out(res.results)



# revision 3
# speedup vs baseline: 1.0500x; 1.0122x over previous
"""MoE feed-forward (shared + top-2 of 8 routed experts), expert-parallel
across 8 trn2 cores.

Sharding strategy (per the spec's expert-parallel hint): the stacked expert
weights [E,d,f] are sharded along E — core c owns expert c. Token dispatch/
combine happens at the shard/unshard boundary on the host: while building
per-core inputs, the host runs the (tiny, 0.1% of FLOPs) router
(softmax -> top-2 -> renormalize) and gathers each expert's tokens into that
core's input shard, padded to a uniform capacity CE (SPMD requires one
program). The shared expert is token-parallel: core c also runs the shared
FFN for tokens [512c, 512(c+1)). Unshard scatters the (device-scaled) expert
rows back and sums with the shared rows; each expert's token list has unique
token ids, so fancy-index += is exact.

Device program per core (identical on all 8):
  - routed:  h^T = Wg^T xg^T, u^T = Wu^T xg^T (bf16, fp32 PSUM), g = silu(h)*u
             y^T = Wd^T g^T   [CE tokens]
  - shared:  same, 512 tokens
Activations keep tokens in the free dim throughout (transposed layout): every
matmul's stationary operand is a natural [128,128] weight tile that streams
all tokens back-to-back (weight loads hide under the previous stream), and
PSUM groups are 3x384 tokens. The host applies the per-token gate weight and
un-transposes during unshard. All DMA is split into <=256KB per-chunk
transfers on the SP HWDGE ring (large single transfers serialize; the ACT
ring proved both slower and rece-prone here).
"""

import numpy as np

E = 8          # routed experts
D = 1024       # hidden
F = 1024       # intermediate
B, S = 2, 2048
T = B * S      # 4096 tokens
NCORES = 8
TCS = T // NCORES  # 512 shared-expert tokens per core
P = 128
DK = D // P    # 8 contraction chunks over D
FT = F // P    # 8 f tiles

_CACHE: dict = {}
_ROUTING: dict = {}


def _psum_groups(n):
    # near-uniform groups of <=512 tokens (uniform sizes keep the weight-load
    # fully hidden under each stream)
    k = -(-n // 512)
    q = -(-n // k)
    out = []
    o = 0
    while o < n:
        g = min(q, n - o)
        out.append((o, g))
        o += g
    return out


def _build_nc(ce, reps=1, loop_reps=0):
    import concourse.mybir as mybir
    import concourse.tile as tile
    from concourse import bacc
    from concourse.bass import ts, ds

    dt = mybir.dt
    f32 = dt.float32
    bf16 = dt.bfloat16
    Alu = mybir.AluOpType
    Act = mybir.ActivationFunctionType


    nc = bacc.Bacc("TRN2", target_bir_lowering=False, debug=False,
                   num_devices=NCORES)

    xr_d = nc.dram_tensor("xrT", [P, DK, ce], bf16, kind="ExternalInput").ap()
    xs_d = nc.dram_tensor("xsT", [P, DK, TCS], bf16, kind="ExternalInput").ap()
    wg_d = nc.dram_tensor("wg", [2, P, DK, F], bf16, kind="ExternalInput").ap()
    wu_d = nc.dram_tensor("wu", [2, P, DK, F], bf16, kind="ExternalInput").ap()
    wd_d = nc.dram_tensor("wd", [2, P, FT, D], bf16, kind="ExternalInput").ap()
    y_d = nc.dram_tensor("y", [DK, P, ce + TCS], bf16,
                         kind="ExternalOutput").ap()

    with tile.TileContext(nc) as tc:
        with (
            tc.tile_pool(name="const", bufs=1) as constp,
            tc.tile_pool(name="wgp", bufs=2) as wgp,
            tc.tile_pool(name="wup", bufs=2) as wup,
            tc.tile_pool(name="wdp", bufs=2) as wdp,
            tc.tile_pool(name="gp", bufs=2) as gp,
            tc.tile_pool(name="yp", bufs=3) as yp,
            tc.tile_pool(name="ydp", bufs=1) as ydp,
            tc.tile_pool(name="php", bufs=8, space="PSUM") as php,
        ):
          import contextlib
          ydt = ydp.tile([P, DK, TCS], bf16, tag="ydt", name="ydt")

          def flush_y():
              for dch in range(DK):
                  nc.sync.dma_start(y_d[dch, :, ds(ce, TCS)], ydt[:, dch])

          loop_cm = (tc.For_i(0, loop_reps, 1) if loop_reps
                     else contextlib.nullcontext())
          with loop_cm:
           for _rep in range(reps):
              xr = constp.tile([P, DK, ce], bf16)
              nc.sync.dma_start(xr[:], xr_d[:])
              xs = constp.tile([P, DK, TCS], bf16)
              nc.sync.dma_start(xs[:], xs_d[:])

              def ffn_gu(e, xa, ntok):
                  # weights for this part, split into per-chunk DMAs
                  wg_sb = wgp.tile([P, DK, F], bf16, tag="wg")
                  for dk in range(DK):
                      if dk == 0:
                          for ft in range(FT):
                              nc.sync.dma_start(wg_sb[:, 0, ts(ft, P)],
                                                wg_d[e, :, 0, ts(ft, P)])
                      else:
                          nc.sync.dma_start(wg_sb[:, dk], wg_d[e, :, dk])
                  wu_sb = wup.tile([P, DK, F], bf16, tag="wu")
                  for dk in range(DK):
                      if dk == 0:
                          for ft in range(FT):
                              nc.sync.dma_start(wu_sb[:, 0, ts(ft, P)],
                                                wu_d[e, :, 0, ts(ft, P)])
                      else:
                          nc.sync.dma_start(wu_sb[:, dk], wu_d[e, :, dk])
                  wd_sb = wdp.tile([P, FT, D], bf16, tag="wd")
                  for fk in range(FT):
                      nc.sync.dma_start(wd_sb[:, fk], wd_d[e, :, fk])

                  g_sb = gp.tile([P, FT, ntok], bf16, tag="g")
                  groups = _psum_groups(ntok)
                  for ft in range(FT):
                      # dk outer: each [128,128] weight tile streams all ntok
                      # rows (across psum groups) back-to-back
                      phs = []
                      for gi in range(len(groups)):
                          ph = php.tile([P, groups[gi][1]], f32, tag="ph")
                          phs.append(ph)
                      for dk in range(DK):
                          for gi, (go, gl) in enumerate(groups):
                              nc.tensor.matmul(
                                  phs[gi][:], wg_sb[:, dk, ts(ft, P)],
                                  xa[:, dk, ds(go, gl)],
                                  start=(dk == 0), stop=(dk == DK - 1),
                              )
                      pus = []
                      for gi in range(len(groups)):
                          pu = php.tile([P, groups[gi][1]], f32, tag="ph")
                          pus.append(pu)
                      for dk in range(DK):
                          for gi, (go, gl) in enumerate(groups):
                              nc.tensor.matmul(
                                  pus[gi][:], wu_sb[:, dk, ts(ft, P)],
                                  xa[:, dk, ds(go, gl)],
                                  start=(dk == 0), stop=(dk == DK - 1),
                              )
                      for gi, (go, gl) in enumerate(groups):
                          nc.scalar.activation(g_sb[:, ft, ds(go, gl)],
                                               phs[gi][:], Act.Silu)
                          nc.vector.tensor_tensor(g_sb[:, ft, ds(go, gl)],
                                                  g_sb[:, ft, ds(go, gl)],
                                                  pus[gi][:], Alu.mult)

                  return g_sb, wd_sb, groups

              def ffn_down(state, ntok, col0, delay):
                  g_sb, wd_sb, groups = state
                  # transposed down: y^T[dch, :, tok] — each Wd lhsT tile
                  # streams all ntok rows (across groups) so its weight load
                  # hides fully under the previous stream
                  for dch in range(DK):
                      pys = []
                      for gi in range(len(groups)):
                          py = php.tile([P, groups[gi][1]], f32, tag="ph")
                          pys.append(py)
                      for fk in range(FT):
                          for gi, (go, gl) in enumerate(groups):
                              nc.tensor.matmul(
                                  pys[gi][:], wd_sb[:, fk, ds(dch * P, P)],
                                  g_sb[:, fk, ds(go, gl)],
                                  start=(fk == 0), stop=(fk == FT - 1),
                              )
                      for gi, (go, gl) in enumerate(groups):
                          if delay:
                              nc.vector.tensor_copy(ydt[:, dch], pys[gi][:])
                          else:
                              # routed-part stores ride the Act HWDGE ring so
                              # the SP ring stays loads-only: next iteration's
                              # x/weight prefetch is never queued behind
                              # stores that are gated on down-phase compute
                              ysb = yp.tile([P, groups[gi][1]], bf16, tag="y")
                              nc.vector.tensor_copy(ysb[:], pys[gi][:])
                              nc.scalar.dma_start(
                                  y_d[dch, :, ds(col0 + go, gl)], ysb[:])

              st0 = ffn_gu(0, xr, ce)
              flush_y()
              st1 = ffn_gu(1, xs, TCS)
              ffn_down(st0, ce, 0, False)
              ffn_down(st1, TCS, ce, True)

          flush_y()

    nc.compile()
    return nc


def _get_nc(reps=1, loop_reps=0):
    ce = _ROUTING["ce"]
    key = (ce, reps, loop_reps)
    if key not in _CACHE:
        _CACHE[key] = _build_nc(ce, reps, loop_reps)
    return _CACHE[key]


def _route(x, gate_w):
    """Host router: softmax -> top-2 (jax top_k tie order) -> renormalize."""
    xf = np.asarray(x, np.float32).reshape(T, D)
    logits = xf @ np.asarray(gate_w, np.float32)
    m = logits.max(-1, keepdims=True)
    q = np.exp(logits - m)
    gate = q / q.sum(-1, keepdims=True)
    order = np.argsort(-gate, axis=-1, kind="stable")
    topi = order[:, :2]
    topw = np.take_along_axis(gate, topi, axis=-1)
    topw = topw / (topw.sum(-1, keepdims=True) + 1e-20)
    return xf, topi, topw


def make_in_maps(x, gate_w, sw_gate, sw_up, sw_down, ew_gate, ew_up, ew_down):
    import ml_dtypes
    bf16 = ml_dtypes.bfloat16

    xf, topi, topw = _route(x, gate_w)

    idxs, ws = [], []
    for e in range(E):
        sel = np.nonzero(topi == e)
        idxs.append(sel[0].astype(np.int64))          # token ids, sorted
        ws.append(topw[sel].astype(np.float32))
    counts = [len(i) for i in idxs]
    ce = max(max(counts), 1)
    _ROUTING.update(ce=ce, idxs=idxs, counts=counts, ws=ws)

    sw = [np.asarray(a, np.float32) for a in (sw_gate, sw_up, sw_down)]
    ew = [np.asarray(a, np.float32) for a in (ew_gate, ew_up, ew_down)]

    def prep_gu(w):   # [D, F] -> [128, DK, F] bf16
        return np.ascontiguousarray(
            w.reshape(DK, P, F).transpose(1, 0, 2).astype(bf16))

    def prep_d(w):    # [F, D] -> [128, FT, D] bf16
        return np.ascontiguousarray(
            w.reshape(FT, P, D).transpose(1, 0, 2).astype(bf16))

    def prep_x(rows, n):  # [n?, D] pad to n -> [128, DK, n] bf16
        xp = np.zeros((n, D), np.float32)
        xp[:len(rows)] = rows
        return np.ascontiguousarray(
            xp.T.reshape(DK, P, n).transpose(1, 0, 2).astype(bf16))

    sg, su, sd = (prep_gu(sw[0]), prep_gu(sw[1]), prep_d(sw[2]))

    in_maps = []
    for c in range(NCORES):
        xg = prep_x(xf[idxs[c]], ce)
        xsl = prep_x(xf[c * TCS:(c + 1) * TCS], TCS)
        in_maps.append({
            "xrT": xg, "xsT": xsl,
            "wg": np.stack([prep_gu(ew[0][c]), sg]),
            "wu": np.stack([prep_gu(ew[1][c]), su]),
            "wd": np.stack([prep_d(ew[2][c]), sd]),
        })
    return in_maps


def assemble_out(results):
    ce = _ROUTING["ce"]
    idxs, counts, ws = _ROUTING["idxs"], _ROUTING["counts"], _ROUTING["ws"]
    y = np.zeros((T, D), np.float32)
    for c in range(NCORES):
        yt = np.asarray(results[c]["y"], np.float32)   # [DK, P, ce+TCS]
        ys = yt[:, :, ce:].transpose(2, 0, 1).reshape(TCS, D)
        y[c * TCS:(c + 1) * TCS] = ys
    for c in range(NCORES):
        yt = np.asarray(results[c]["y"], np.float32)
        n = counts[c]
        yr = yt[:, :, :n].transpose(2, 0, 1).reshape(n, D)
        y[idxs[c]] += yr * ws[c][:, None]
    return y.reshape(B, S, D)


def kernel(x, gate_w, sw_gate, sw_up, sw_down, ew_gate, ew_up, ew_down):
    from concourse.bass_utils import run_bass_kernel_spmd

    in_maps = make_in_maps(x, gate_w, sw_gate, sw_up, sw_down,
                           ew_gate, ew_up, ew_down)
    nc = _get_nc()
    res = run_bass_kernel_spmd(nc, in_maps, list(range(NCORES)))
    return assemble_out(res.results)



# revision 4
# speedup vs baseline: 1.0674x; 1.0166x over previous
"""MoE feed-forward (shared + top-2 of 8 routed experts), expert-parallel
across 8 trn2 cores.

Sharding strategy (per the spec's expert-parallel hint): the stacked expert
weights [E,d,f] are sharded along E — core c owns expert c. Token dispatch/
combine happens at the shard/unshard boundary on the host: while building
per-core inputs, the host runs the (tiny, 0.1% of FLOPs) router
(softmax -> top-2 -> renormalize) and gathers each expert's tokens into that
core's input shard, padded to a uniform capacity CE (SPMD requires one
program). The shared expert is token-parallel: core c also runs the shared
FFN for tokens [512c, 512(c+1)). Unshard scatters the (device-scaled) expert
rows back and sums with the shared rows; each expert's token list has unique
token ids, so fancy-index += is exact.

Device program per core (identical on all 8):
  - routed:  h^T = Wg^T xg^T, u^T = Wu^T xg^T (bf16, fp32 PSUM), g = silu(h)*u
             y^T = Wd^T g^T   [CE tokens]
  - shared:  same, 512 tokens
Activations keep tokens in the free dim throughout (transposed layout): every
matmul's stationary operand is a natural [128,128] weight tile that streams
all tokens back-to-back (weight loads hide under the previous stream), and
PSUM groups are 3x384 tokens. The host applies the per-token gate weight and
un-transposes during unshard. All DMA is split into <=256KB per-chunk
transfers on the SP HWDGE ring (large single transfers serialize; the ACT
ring proved both slower and rece-prone here).
"""

import numpy as np

E = 8          # routed experts
D = 1024       # hidden
F = 1024       # intermediate
B, S = 2, 2048
T = B * S      # 4096 tokens
NCORES = 8
TCS = T // NCORES  # 512 shared-expert tokens per core
P = 128
DK = D // P    # 8 contraction chunks over D
FT = F // P    # 8 f tiles

_CACHE: dict = {}
_ROUTING: dict = {}


def _psum_groups(n):
    # near-uniform groups of <=512 tokens (uniform sizes keep the weight-load
    # fully hidden under each stream)
    k = -(-n // 512)
    q = -(-n // k)
    out = []
    o = 0
    while o < n:
        g = min(q, n - o)
        out.append((o, g))
        o += g
    return out


def _build_nc(ce, reps=1, loop_reps=0):
    import concourse.mybir as mybir
    import concourse.tile as tile
    from concourse import bacc
    from concourse.bass import ts, ds

    dt = mybir.dt
    f32 = dt.float32
    bf16 = dt.bfloat16
    Alu = mybir.AluOpType
    Act = mybir.ActivationFunctionType


    nc = bacc.Bacc("TRN2", target_bir_lowering=False, debug=False,
                   num_devices=NCORES)

    xr_d = nc.dram_tensor("xrT", [P, DK, ce], bf16, kind="ExternalInput").ap()
    xs_d = nc.dram_tensor("xsT", [P, DK, TCS], bf16, kind="ExternalInput").ap()
    wg_d = nc.dram_tensor("wg", [2, P, FT, D], bf16, kind="ExternalInput").ap()
    wu_d = nc.dram_tensor("wu", [2, P, FT, D], bf16, kind="ExternalInput").ap()
    wd_d = nc.dram_tensor("wd", [2, P, FT, D], bf16, kind="ExternalInput").ap()
    y_d = nc.dram_tensor("y", [DK, P, ce + TCS], bf16,
                         kind="ExternalOutput").ap()

    with tile.TileContext(nc) as tc:
        with (
            tc.tile_pool(name="const", bufs=1) as constp,
            tc.tile_pool(name="wgp", bufs=2) as wgp,
            tc.tile_pool(name="wup", bufs=2) as wup,
            tc.tile_pool(name="wdp", bufs=2) as wdp,
            tc.tile_pool(name="gp", bufs=2) as gp,
            tc.tile_pool(name="yp", bufs=3) as yp,
            tc.tile_pool(name="ydp", bufs=1) as ydp,
            tc.tile_pool(name="php", bufs=8, space="PSUM") as php,
        ):
          import contextlib
          ydt = ydp.tile([P, DK, TCS], bf16, tag="ydt", name="ydt")

          def flush_y():
              for dch in range(DK):
                  nc.sync.dma_start(y_d[dch, :, ds(ce, TCS)], ydt[:, dch])

          loop_cm = (tc.For_i(0, loop_reps, 1) if loop_reps
                     else contextlib.nullcontext())
          with loop_cm:
           for _rep in range(reps):
              xr = constp.tile([P, DK, ce], bf16)
              nc.sync.dma_start(xr[:], xr_d[:])
              xs = constp.tile([P, DK, TCS], bf16)
              nc.sync.dma_start(xs[:], xs_d[:])

              def ffn_gu(e, xa, ntok):
                  # ft-major weight layout: DMA chunk per ft tile so arrival
                  # order matches the ft-outer consumption order exactly
                  wg_sb = wgp.tile([P, FT, D], bf16, tag="wg")
                  for ft in range(FT):
                      nc.sync.dma_start(wg_sb[:, ft], wg_d[e, :, ft])
                  wu_sb = wup.tile([P, FT, D], bf16, tag="wu")
                  for ft in range(FT):
                      nc.sync.dma_start(wu_sb[:, ft], wu_d[e, :, ft])
                  wd_sb = wdp.tile([P, FT, D], bf16, tag="wd")
                  for fk in range(FT):
                      nc.sync.dma_start(wd_sb[:, fk], wd_d[e, :, fk])

                  g_sb = gp.tile([P, FT, ntok], bf16, tag="g")
                  groups = _psum_groups(ntok)
                  for ft in range(FT):
                      # dk outer: each [128,128] weight tile streams all ntok
                      # rows (across psum groups) back-to-back
                      phs = []
                      for gi in range(len(groups)):
                          ph = php.tile([P, groups[gi][1]], f32, tag="ph")
                          phs.append(ph)
                      for dk in range(DK):
                          for gi, (go, gl) in enumerate(groups):
                              nc.tensor.matmul(
                                  phs[gi][:], wg_sb[:, ft, ds(dk * P, P)],
                                  xa[:, dk, ds(go, gl)],
                                  start=(dk == 0), stop=(dk == DK - 1),
                              )
                      pus = []
                      for gi in range(len(groups)):
                          pu = php.tile([P, groups[gi][1]], f32, tag="ph")
                          pus.append(pu)
                      for dk in range(DK):
                          for gi, (go, gl) in enumerate(groups):
                              nc.tensor.matmul(
                                  pus[gi][:], wu_sb[:, ft, ds(dk * P, P)],
                                  xa[:, dk, ds(go, gl)],
                                  start=(dk == 0), stop=(dk == DK - 1),
                              )
                      for gi, (go, gl) in enumerate(groups):
                          nc.scalar.activation(g_sb[:, ft, ds(go, gl)],
                                               phs[gi][:], Act.Silu)
                          nc.vector.tensor_tensor(g_sb[:, ft, ds(go, gl)],
                                                  g_sb[:, ft, ds(go, gl)],
                                                  pus[gi][:], Alu.mult)

                  return g_sb, wd_sb, groups

              def ffn_down(state, ntok, col0, delay):
                  g_sb, wd_sb, groups = state
                  # transposed down: y^T[dch, :, tok] — each Wd lhsT tile
                  # streams all ntok rows (across groups) so its weight load
                  # hides fully under the previous stream
                  for dch in range(DK):
                      pys = []
                      for gi in range(len(groups)):
                          py = php.tile([P, groups[gi][1]], f32, tag="ph")
                          pys.append(py)
                      for fk in range(FT):
                          for gi, (go, gl) in enumerate(groups):
                              nc.tensor.matmul(
                                  pys[gi][:], wd_sb[:, fk, ds(dch * P, P)],
                                  g_sb[:, fk, ds(go, gl)],
                                  start=(fk == 0), stop=(fk == FT - 1),
                              )
                      for gi, (go, gl) in enumerate(groups):
                          if delay:
                              nc.vector.tensor_copy(ydt[:, dch], pys[gi][:])
                          else:
                              # routed-part stores ride the Act HWDGE ring so
                              # the SP ring stays loads-only: next iteration's
                              # x/weight prefetch is never queued behind
                              # stores that are gated on down-phase compute
                              ysb = yp.tile([P, groups[gi][1]], bf16, tag="y")
                              nc.vector.tensor_copy(ysb[:], pys[gi][:])
                              nc.scalar.dma_start(
                                  y_d[dch, :, ds(col0 + go, gl)], ysb[:])

              st0 = ffn_gu(0, xr, ce)
              flush_y()
              st1 = ffn_gu(1, xs, TCS)
              ffn_down(st0, ce, 0, False)
              ffn_down(st1, TCS, ce, True)

          flush_y()

    nc.compile()
    return nc


def _get_nc(reps=1, loop_reps=0):
    ce = _ROUTING["ce"]
    key = (ce, reps, loop_reps)
    if key not in _CACHE:
        _CACHE[key] = _build_nc(ce, reps, loop_reps)
    return _CACHE[key]


def _route(x, gate_w):
    """Host router: softmax -> top-2 (jax top_k tie order) -> renormalize."""
    xf = np.asarray(x, np.float32).reshape(T, D)
    logits = xf @ np.asarray(gate_w, np.float32)
    m = logits.max(-1, keepdims=True)
    q = np.exp(logits - m)
    gate = q / q.sum(-1, keepdims=True)
    order = np.argsort(-gate, axis=-1, kind="stable")
    topi = order[:, :2]
    topw = np.take_along_axis(gate, topi, axis=-1)
    topw = topw / (topw.sum(-1, keepdims=True) + 1e-20)
    return xf, topi, topw


def make_in_maps(x, gate_w, sw_gate, sw_up, sw_down, ew_gate, ew_up, ew_down):
    import ml_dtypes
    bf16 = ml_dtypes.bfloat16

    xf, topi, topw = _route(x, gate_w)

    idxs, ws = [], []
    for e in range(E):
        sel = np.nonzero(topi == e)
        idxs.append(sel[0].astype(np.int64))          # token ids, sorted
        ws.append(topw[sel].astype(np.float32))
    counts = [len(i) for i in idxs]
    ce = max(max(counts), 1)
    _ROUTING.update(ce=ce, idxs=idxs, counts=counts, ws=ws)

    sw = [np.asarray(a, np.float32) for a in (sw_gate, sw_up, sw_down)]
    ew = [np.asarray(a, np.float32) for a in (ew_gate, ew_up, ew_down)]

    def prep_gu(w):   # [D, F] -> [128, FT, D] bf16, ft-major
        return np.ascontiguousarray(
            w.reshape(DK, P, FT, P).transpose(1, 2, 0, 3)
            .reshape(P, FT, D).astype(bf16))

    def prep_d(w):    # [F, D] -> [128, FT, D] bf16
        return np.ascontiguousarray(
            w.reshape(FT, P, D).transpose(1, 0, 2).astype(bf16))

    def prep_x(rows, n):  # [n?, D] pad to n -> [128, DK, n] bf16
        xp = np.zeros((n, D), np.float32)
        xp[:len(rows)] = rows
        return np.ascontiguousarray(
            xp.T.reshape(DK, P, n).transpose(1, 0, 2).astype(bf16))

    sg, su, sd = (prep_gu(sw[0]), prep_gu(sw[1]), prep_d(sw[2]))

    in_maps = []
    for c in range(NCORES):
        xg = prep_x(xf[idxs[c]], ce)
        xsl = prep_x(xf[c * TCS:(c + 1) * TCS], TCS)
        in_maps.append({
            "xrT": xg, "xsT": xsl,
            "wg": np.stack([prep_gu(ew[0][c]), sg]),
            "wu": np.stack([prep_gu(ew[1][c]), su]),
            "wd": np.stack([prep_d(ew[2][c]), sd]),
        })
    return in_maps


def assemble_out(results):
    ce = _ROUTING["ce"]
    idxs, counts, ws = _ROUTING["idxs"], _ROUTING["counts"], _ROUTING["ws"]
    y = np.zeros((T, D), np.float32)
    for c in range(NCORES):
        yt = np.asarray(results[c]["y"], np.float32)   # [DK, P, ce+TCS]
        ys = yt[:, :, ce:].transpose(2, 0, 1).reshape(TCS, D)
        y[c * TCS:(c + 1) * TCS] = ys
    for c in range(NCORES):
        yt = np.asarray(results[c]["y"], np.float32)
        n = counts[c]
        yr = yt[:, :, :n].transpose(2, 0, 1).reshape(n, D)
        y[idxs[c]] += yr * ws[c][:, None]
    return y.reshape(B, S, D)


def kernel(x, gate_w, sw_gate, sw_up, sw_down, ew_gate, ew_up, ew_down):
    from concourse.bass_utils import run_bass_kernel_spmd

    in_maps = make_in_maps(x, gate_w, sw_gate, sw_up, sw_down,
                           ew_gate, ew_up, ew_down)
    nc = _get_nc()
    res = run_bass_kernel_spmd(nc, in_maps, list(range(NCORES)))
    return assemble_out(res.results)

